# revision 21
# baseline (speedup 1.0000x reference)
"""Trainium2 Bass kernel for 16-head cross attention, tensor-parallel over 8 cores.

Reference computation (fp32):
    q = (x @ Wq).reshape(n, 16, 64)   # x [2048, 1024], Wq [1024, 1024]
    k = (ctx @ Wk).reshape(m, 16, 64) # ctx [2048, 768]
    v = (ctx @ Wv).reshape(m, 16, 64)
    out[h] = softmax(q[h] @ k[h].T / 8) @ v[h]
    y = out.reshape(n, 1024) @ Wo
Sharding: heads split 2-per-core (columns of Wq/Wk/Wv, rows of Wo). Each core
produces a partial y (blocked layout, bf16); the host sums the 8 partials.

v5 design (prev best 126.5us was exp/Scalar-paced at ~1067ns per mt):
  - Scores are computed transposed (scoresT [m, n]); softmax denominators come
    from a ones-column in v; exp has no max subtraction. All scores are
    pre-scaled by 128*log2e (folded into Wq host-side) so both exp paths read
    the same psum: a global 2^65 factor (B=24576 bits) cancels in softmax.
  - exp is SPLIT per-mt between two engines: Scalar runs exact ACT exp
    (scale=1/184.67, bias=65*ln2); the Vector engine runs EXP2_BITS_ANT, a
    custom one-pass DVE op: t = s'+B; f = mantissa-extract via AND/OR bit
    masks (t in [2^14,2^15) so the exponent field is constant); out_i16 =
    t + f*(C2 - f*P2), a quadratic mantissa correction (max 0.64% rel err,
    vs 0.39% for the exact bf16 path). The int16 result IS bf16 bits.
  - The two score matmuls (K=64 per head) are packed into concurrent PE
    row-tiles via tile_position (0,0)/(64,0), separate psum banks.
  - Normalization: reciprocal_approx_fast on the psum ones-row, gpsimd
    partition_broadcast, then one scalar_tensor_tensor (psum x bcast -> bf16
    oT) -- no separate evacuation copy.
  - DMA: host packs x/ctx into [piece, 128, k, 512] blobs so each dma_start
    moves 0.75-1MiB with 6-8KB contiguous per-partition runs (~300GB/s vs
    ~60GB/s for the old strided pieces). Inputs stream on the sync queue in
    need-order; outputs are staged per-block into a [128, 8, 512] tile and
    written with one gpsimd DMA per block.
  - PSUM: score ping-pong 2x2 banks + PV 2 + aux(v/proj) 1 + emit(kT/qT) 1.
"""

import os
import sys

for _p in ("/opt/trn_rl_repo", "/root/.axon_site/_ro/trn_rl_repo"):
    if os.path.isdir(_p) and _p not in sys.path:
        sys.path.insert(0, _p)

import numpy as np
import ml_dtypes

import concourse.bass as bass
import concourse.mybir as mybir
import concourse.tile as tile
from concourse import bacc
from concourse.bass_utils import run_bass_kernel_spmd

P = 128
N_TOK = 2048
M_TOK = 2048
D = 1024
C = 768
DH = 64
NB = 512
BLOCKS = [(0, 512), (512, 512), (1024, 512), (1536, 512)]
DK = D // P   # 8 x-contraction chunks
CK = C // P   # 6 ctx-contraction chunks
MT = M_TOK // P  # 16 context chunks
NPC = 4       # 512-wide DMA pieces per tensor
AT_LEAD = 2   # PV trails scores by this many mt iterations

# ---- exp scaling: score psum s' = s * 128*log2e; at = 2^65 * e^s ----
L2E = 1.4426950408889634
A_SCALE = 128.0 * L2E            # folded into Wq on host
ACT_SCALE = 1.0 / A_SCALE        # Scalar ACT: exp(s'/A + 65 ln2)
ACT_BIAS = 65.0 * float(np.log(2.0))
# EXP2_BITS_ANT constants (fit offline, see docstring)
EXP2_B = 24576.506538311176
EXP2_C2 = -5743.76031008344
EXP2_P2 = -718603.0723100811
EXP2_MASK = float(np.int32(0x0000FFFF).view(np.float32))  # mantissa-low mask

# per-16-mt engine assignment: which mts the Vector engine exps (rest Scalar)
N_VMT = int(os.environ.get("CA_NVMT", "0"))
V_SET = set(np.linspace(0, MT - 1, N_VMT, dtype=int).tolist()) if N_VMT else set()
PACK_SCORES = os.environ.get("CA_PACK", "1") == "1"
EXP_OP_MODE = os.environ.get("CA_EXPOP", "exp2")

DT = mybir.dt.bfloat16
NP_DT = ml_dtypes.bfloat16

# ---- register the custom DVE op (one pass, 8 ALU slices) ----
_EXP2_OP = None


def _register_exp2_op():
    global _EXP2_OP
    if _EXP2_OP is not None:
        return _EXP2_OP
    from concourse import dve_ops
    from concourse.dve_spec import Spec, Bin, Src0, Src1, C0, C1, C2, One, lower
    from concourse.dve_spec import AluOp
    from concourse.dve_uop import DveOpSpec

    _t = Src0 + C0
    _a = Bin(AluOp.BITWISE_AND, _t, C1)
    _b = Bin(AluOp.BITWISE_OR, _a, One)
    _w = _b - One
    _e = _w * Src1
    _h = C2 - _e
    _body = _t + (_w * _h)

    def _ref(in0, in1, s0, s1, imm2):
        t = (in0.astype(np.float32) + np.float32(s0)).astype(np.float32)
        mask = np.float32(s1).view(np.int32)
        a = (t.view(np.int32) & mask).view(np.float32)
        b = (a.view(np.int32) | np.float32(1.0).view(np.int32)).view(np.float32)
        w = (b - np.float32(1.0)).astype(np.float32)
        e = (w * in1.astype(np.float32)).astype(np.float32)
        h = (np.float32(imm2) - e).astype(np.float32)
        return t + w * h

    name = "EXP2_BITS_ANT"
    spec = Spec(body=_body, reference=_ref)
    if name not in dve_ops._SUB_OPCODE_FOR_NAME:
        row = max(dve_ops._SUB_OPCODE_FOR_NAME.values()) + 1
        assert row < 0x20
        dve_ops._SUB_OPCODE_FOR_NAME[name] = row
    shas = {}
    for ver in ("v3", "v4"):
        uops = lower(spec, ver=ver)
        shas[ver] = DveOpSpec(
            name=name,
            opcode=dve_ops._SUB_OPCODE_FOR_NAME[name],
            uops=uops,
            rd1_en=True,
        ).sha(ver)
    op = dve_ops.DveOp(name, spec, subdim=False, uops_sha=shas)
    if all(o.name != name for o in dve_ops.OPS):
        dve_ops.OPS.append(op)
    dve_ops.CUSTOM_DVE_SPECS[name] = spec
    _EXP2_OP = op
    return op


def build_core_program():
    f32 = mybir.dt.float32
    exp2_op = _register_exp2_op()

    nc = bacc.Bacc("TRN2", target_bir_lowering=False, debug=False)

    # host-packed blobs: [piece, 128, contraction-chunk, 512]
    xb = nc.declare_dram_parameter("xb", [NPC, P, DK, NB], DT, isOutput=False)
    cb = nc.declare_dram_parameter("cb", [NPC, P, CK, NB], DT, isOutput=False)
    wq = nc.declare_dram_parameter("wq", [P, DK, P], DT, isOutput=False)
    wk = nc.declare_dram_parameter("wk", [P, CK, P], DT, isOutput=False)
    wv = nc.declare_dram_parameter("wv", [P, CK, P], DT, isOutput=False)
    wo = nc.declare_dram_parameter("wo", [P, D], DT, isOutput=False)
    # output blob: [block, 128, slab, 512]; y[n0+c, s*128+p] = yO[b, p, s, c]
    yO = nc.declare_dram_parameter("yO", [len(BLOCKS), P, 8, NB], DT, isOutput=True)

    with tile.TileContext(nc) as tc:
        with (
            tc.tile_pool(name="wts", bufs=1) as wts,
            tc.tile_pool(name="att", bufs=4) as att,
            tc.tile_pool(name="yout", bufs=2) as yout,
            tc.tile_pool(name="small", bufs=2) as small,
            tc.tile_pool(name="ps_sc", bufs=2, space="PSUM") as ps_sc,   # 2x2
            tc.tile_pool(name="ps_pv", bufs=2, space="PSUM") as ps_pv,   # 2x1
            tc.tile_pool(name="ps_aux", bufs=1, space="PSUM") as ps_aux,  # 1
            tc.tile_pool(name="ps_emit", bufs=1, space="PSUM") as ps_emit,  # 1
        ):
            # ---- input DMA in need-order, all on the sync (HWDGE) queue ----
            wk_sb = wts.tile([P, CK, P], DT)
            nc.sync.dma_start(wk_sb[:], wk.ap())
            xT_sb = wts.tile([P, NPC, DK, NB], DT)
            ctx_sb = wts.tile([P, NPC, CK, NB], DT)
            nc.sync.dma_start(ctx_sb[:, 0], cb.ap()[0])
            wq_sb = wts.tile([P, DK, P], DT)
            nc.sync.dma_start(wq_sb[:], wq.ap())
            nc.sync.dma_start(xT_sb[:, 0], xb.ap()[0])
            wv_sb = wts.tile([P, CK, P], DT)
            nc.sync.dma_start(wv_sb[:], wv.ap())

            # ACT exp-table preload hides under the input DMA
            warm = small.tile([1, 8], f32, tag="warm", bufs=1)
            nc.vector.memset(warm[:], 0.0)
            nc.scalar.activation(warm[:], warm[:], mybir.ActivationFunctionType.Exp)

            nc.sync.dma_start(ctx_sb[:, 1], cb.ap()[1])
            nc.sync.dma_start(xT_sb[:, 1], xb.ap()[1])
            nc.sync.dma_start(ctx_sb[:, 2], cb.ap()[2])
            nc.sync.dma_start(ctx_sb[:, 3], cb.ap()[3])
            nc.sync.dma_start(xT_sb[:, 2], xb.ap()[2])
            nc.sync.dma_start(xT_sb[:, 3], xb.ap()[3])
            wo_sb = wts.tile([P, D], DT)
            nc.sync.dma_start(wo_sb[:], wo.ap())

            # ---- persistent intermediates ----
            kT_sb = wts.tile([P, M_TOK], DT)   # [dh(2 heads), m]
            qT_sb = wts.tile([P, N_TOK], DT)   # [dq(2 heads), n]
            # v layout [m, mt, head, 128]: col 0 = ones (softmax denominator
            # lands on psum partition 0), cols 64..127 = v values
            vAB = wts.tile([P, MT, 2, P], DT)
            oT_sb = wts.tile([P, N_TOK], DT)   # attn out^T, both heads
            p2c = wts.tile([P, 1], f32)        # EXP2 Src1 per-partition const
            nc.vector.memset(p2c[:], EXP2_P2)
            abias = wts.tile([P, 1], f32)      # ACT exp bias (65*ln2)
            nc.vector.memset(abias[:], ACT_BIAS)
            z1c = wts.tile([P, 1], f32)
            nc.vector.memset(z1c[:], 0.0)

            nc.vector.memset(vAB[:], 0.0)
            nc.vector.memset(vAB[:, :, :, 0:1], 1.0)

            def mm(out, lhsT, rhs, start, stop, tpos=None):
                nc.tensor.matmul(
                    out, lhsT, rhs, start=start, stop=stop, tile_position=tpos
                )

            # ---- emission helpers ----
            def emit_kT_group(g):
                # kT for one 512-wide m group: 6 accumulating N=512 matmuls
                ps = ps_emit.tile([P, NB], f32, tag="emit", name="ps_kg")
                for ck in range(CK):
                    mm(ps, wk_sb[:, ck, :], ctx_sb[:, g, ck, :],
                       start=(ck == 0), stop=(ck == CK - 1))
                nc.vector.tensor_copy(kT_sb[:, g * NB:(g + 1) * NB], ps)

            emit_ps = {}

            def qT_step(j, lo, hi):
                n0, w = BLOCKS[j]
                if lo == 0:
                    emit_ps[j] = ps_emit.tile([P, NB], f32, tag="emit",
                                              name=f"ps_q{j}")
                ps = emit_ps[j][:, :w]
                for c in range(lo, hi):
                    mm(ps, wq_sb[:, c, :], xT_sb[:, j, c, :w],
                       start=(c == 0), stop=(c == DK - 1))
                if hi == DK:
                    nc.vector.tensor_copy(qT_sb[:, n0:n0 + w], ps)

            def emit_v(mt):
                g, off = divmod(mt, 4)
                ps = ps_aux.tile([P, NB], f32, tag="aux", name="ps_v")[:, :P]
                for ck in range(CK):
                    mm(ps, ctx_sb[:, g, ck, off * P:(off + 1) * P], wv_sb[:, ck, :],
                       start=(ck == 0), stop=(ck == CK - 1))
                # one fused copy: [128, 2, 64] into both heads' value slots
                nc.vector.tensor_copy(
                    vAB[:, mt, :, DH:P],
                    ps.rearrange("p (h d) -> p h d", h=2),
                )

            def emit_proj(s, j, eng, force_split=False, alt=False):
                # one 128-row slab of y for block j into the staging tile;
                # alt=True ping-pongs between the aux and emit banks so the
                # next slab's matmul overlaps this slab's copy
                n0, w = BLOCKS[j]
                pool = ps_emit if (alt and s % 2 == 1) else ps_aux
                tag = "emit" if (alt and s % 2 == 1) else "aux"
                ps = pool.tile([P, NB], f32, tag=tag, name="ps_proj")[:, :w]
                mm(ps, wo_sb[:, s * P:(s + 1) * P], oT_sb[:, n0:n0 + w],
                   start=True, stop=True)
                if eng is nc.scalar and (N_VMT > 0 or force_split):
                    nc.scalar.copy(ystage[j][:, s, :w], ps)
                else:
                    nc.vector.tensor_copy(ystage[j][:, s, :w], ps)

            ystage = {}

            def exp_tile(mt, sc, at, w):
                if mt in V_SET:
                    # custom op writes the bf16-bits VALUE as f32 (custom-DVE
                    # int16 output hangs the engine); a native tensor_scalar
                    # does the f32 -> int16 RNE convert.
                    esc = small.tile([P, 2 * NB], f32, tag="esc", name="esc",
                                     bufs=2)
                    nc.vector._custom_dve(
                        exp2_op,
                        out=esc[:, : 2 * w],
                        in0=sc[:, :, :w].rearrange("p h n -> p (h n)"),
                        in1=p2c[:],
                        s0=EXP2_B,
                        s1=EXP2_MASK,
                        imm2=EXP2_C2,
                    )
                    nc.vector.tensor_scalar(
                        at[:, :, :w].rearrange("p h n -> p (h n)"),
                        esc[:, : 2 * w],
                        0.0, None, mybir.AluOpType.add,
                    )
                else:
                    nc.scalar.activation(
                        at[:, :, :w], sc[:, :, :w],
                        mybir.ActivationFunctionType.Exp,
                        bias=abias[:], scale=ACT_SCALE,
                    )

            def normalize_pair(pvA, pvB, nsl, w):
                # both recips first (V), then both broadcasts (G), then both
                # muls (V) -- the two chains overlap across engines
                rcs, bcss = [], []
                for pv, tag in ((pvA, "bcsA"), (pvB, "bcsB")):
                    rcf = small.tile([1, NB], f32, tag="recip", name="rcf")
                    nc.vector.reciprocal_approx_fast(rcf[:, :w], pv[0:1, :w])
                    rcs.append(rcf)
                for rcf, tag in zip(rcs, ("bcsA", "bcsB")):
                    bcs = small.tile([DH, NB], f32, tag=tag, name="bcs")
                    nc.gpsimd.partition_broadcast(bcs[:, :w], rcf[:, :w])
                    bcss.append(bcs)
                for h, (pv, bcs) in enumerate(zip((pvA, pvB), bcss)):
                    nc.vector.scalar_tensor_tensor(
                        oT_sb[h * DH:(h + 1) * DH, nsl],
                        pv[DH:P, :w], 1.0, bcs[:, :w],
                        op0=mybir.AluOpType.mult, op1=mybir.AluOpType.mult,
                    )

            # per-(nb, mt) extra PE work
            extras = {}

            def add_extra(nb, mt, fn):
                extras.setdefault((nb, mt), []).append(fn)

            # block 0: JIT kT group g at mt=4(g-1) (group 0 in prologue);
            # qT(1) spread over mts 12..15 once the emit bank is free
            for g in (1, 2, 3):
                add_extra(0, 4 * (g - 1), lambda g=g: emit_kT_group(g))
            for i, (lo, hi) in enumerate(((0, 2), (2, 4), (4, 6), (6, 8))):
                add_extra(0, 12 + i, lambda lo=lo, hi=hi: qT_step(1, lo, hi))
            # blocks 1-2 produce qT(nb+1) spread over mts 8..11
            for nbb in (1, 2):
                for i, (lo, hi) in enumerate(((0, 2), (2, 4), (4, 6), (6, 8))):
                    add_extra(nbb, 8 + i,
                              lambda j=nbb + 1, lo=lo, hi=hi: qT_step(j, lo, hi))
            # blocks 1-3 run the previous block's Wo projection at mts 6..13
            for nbb in (1, 2, 3):
                for s in range(8):
                    eng = nc.scalar if s % 2 == 0 else nc.vector
                    add_extra(nbb, 6 + s,
                              lambda s=s, j=nbb - 1, e=eng, a=(nbb == 3):
                              emit_proj(s, j, e, alt=a))

            # ---- PE warmup: junk matmuls under the DMA window ----
            with nc.named_scope("warmup"):
                wps = ps_emit.tile([P, NB], f32, tag="emit", name="wps")
                for i in range(36):
                    mm(wps[:, :P], wk_sb[:, i % CK, :], wk_sb[:, (i + 1) % CK, :],
                       start=(i == 0), stop=(i == 35))

            # ---- prologue: kT group 0, then qT(0) ----
            with nc.named_scope("prologue"):
                emit_kT_group(0)
                qT_step(0, 0, DK)

            # ---- attention blocks ----
            def emit_pv(pvA, pvB, at, j, w):
                st, sp = (j == 0), (j == MT - 1)
                atA, atB = at[:, 0, :w], at[:, 1, :w]
                if at.dtype != DT:
                    atA, atB = atA.bitcast(DT), atB.bitcast(DT)
                mm(pvA[:, :w], vAB[:, j, 0, :], atA, start=st, stop=sp)
                mm(pvB[:, :w], vAB[:, j, 1, :], atB, start=st, stop=sp)

            for nb, (n0, w) in enumerate(BLOCKS):
                nsl = slice(n0, n0 + w)
                last = nb == len(BLOCKS) - 1
                ystage[nb] = yout.tile([P, 8, NB], DT, tag="yout",
                                       name=f"yst{nb}")
                with nc.named_scope(f"att{nb}"):
                    pvA = ps_pv.tile([P, NB], f32, tag="pv", name="pvA")
                    pvB = ps_pv.tile([P, NB], f32, tag="pv", name="pvB")
                    at_ring = {}
                    for mt in range(MT):
                        msl = slice(mt * P, (mt + 1) * P)
                        sc = ps_sc.tile([P, 2, NB], f32, tag="sc", name="sc")
                        tpA = (0, 0) if PACK_SCORES else None
                        tpB = (64, 0) if PACK_SCORES else None
                        mm(sc[:, 0, :w], kT_sb[0:DH, msl], qT_sb[0:DH, nsl],
                           start=True, stop=True, tpos=tpA)
                        mm(sc[:, 1, :w], kT_sb[DH:P, msl], qT_sb[DH:P, nsl],
                           start=True, stop=True, tpos=tpB)
                        at_dt = mybir.dt.int16 if mt in V_SET else DT
                        at = att.tile([P, 2, NB], at_dt, tag="at", name="at")
                        exp_tile(mt, sc, at, w)
                        at_ring[mt] = at
                        if nb == 0:
                            emit_v(mt)
                        for fn in extras.get((nb, mt), ()):
                            fn()
                        j = mt - AT_LEAD
                        if j >= 0:
                            emit_pv(pvA, pvB, at_ring.pop(j), j, w)
                    for j in range(MT - AT_LEAD, MT):
                        emit_pv(pvA, pvB, at_ring.pop(j), j, w)
                    normalize_pair(pvA, pvB, nsl, w)
                # block nb-1's projection ran inside block nb (extras);
                # issue its output DMA now
                if nb >= 1:
                    nc.sync.dma_start(yO.ap()[nb - 1], ystage[nb - 1][:])
                if last:
                    with nc.named_scope("tail"):
                        for s in range(8):
                            eng = nc.scalar if s % 2 == 0 else nc.vector
                            emit_proj(s, nb, eng, force_split=True, alt=True)
                            if s % 2 == 1:
                                nc.sync.dma_start(
                                    yO.ap()[nb, :, s - 1:s + 1],
                                    ystage[nb][:, s - 1:s + 1],
                                )

    nc.compile()
    return nc


_NC_CACHE = {}


def _get_nc():
    key = "v5"
    if key not in _NC_CACHE:
        _NC_CACHE[key] = build_core_program()
    return _NC_CACHE[key]


def _shuffle_w(w):
    # [o*P + p, e] -> [p, o, e] so each SBUF partition's rows are contiguous
    o_n = w.shape[0] // P
    return np.ascontiguousarray(
        w.reshape(o_n, P, w.shape[1]).transpose(1, 0, 2)
    )


def _prep_in_maps(x, ctx, Wq, Wk, Wv, Wo):
    # x [n, d] -> xb [piece, p, dk, 512]: xb[g, p, k, c] = x[g*512+c, k*128+p]
    xb = np.ascontiguousarray(
        x.reshape(NPC, NB, DK, P).transpose(0, 3, 2, 1)
    ).astype(NP_DT)
    cbb = np.ascontiguousarray(
        ctx.reshape(NPC, NB, CK, P).transpose(0, 3, 2, 1)
    ).astype(NP_DT)
    Wq_s = (Wq * (A_SCALE / 8.0)).astype(np.float32)
    in_maps = []
    for cc in range(8):
        csl = slice(cc * P, (cc + 1) * P)
        in_maps.append(
            {
                "xb": xb,
                "cb": cbb,
                "wq": _shuffle_w(np.ascontiguousarray(Wq_s[:, csl])).astype(NP_DT),
                "wk": _shuffle_w(np.ascontiguousarray(Wk[:, csl])).astype(NP_DT),
                "wv": _shuffle_w(np.ascontiguousarray(Wv[:, csl])).astype(NP_DT),
                "wo": np.ascontiguousarray(Wo[csl, :]).astype(NP_DT),
            }
        )
    return in_maps


def run(x, ctx, Wq, Wk, Wv, Wo, trace=False):
    nc = _get_nc()
    in_maps = _prep_in_maps(x, ctx, Wq, Wk, Wv, Wo)
    res = run_bass_kernel_spmd(nc, in_maps, core_ids=list(range(8)), trace=trace)
    acc = np.zeros((N_TOK, D), np.float32)
    for r in res.results:
        yb = np.asarray(r["yO"], dtype=np.float32)  # [4, 128, 8, 512]
        acc += yb.transpose(0, 3, 2, 1).reshape(N_TOK, D)
    return acc, res


def kernel(x, ctx, Wq, Wk, Wv, Wo):
    x = np.asarray(x, dtype=np.float32)
    ctx = np.asarray(ctx, dtype=np.float32)
    Wq = np.asarray(Wq, dtype=np.float32)
    Wk = np.asarray(Wk, dtype=np.float32)
    Wv = np.asarray(Wv, dtype=np.float32)
    Wo = np.asarray(Wo, dtype=np.float32)
    y, _ = run(x, ctx, Wq, Wk, Wv, Wo, trace=False)
    return y


# revision 22
# speedup vs baseline: 1.1635x; 1.1635x over previous
"""Trainium2 Bass kernel for 16-head cross attention, tensor-parallel over 8 cores.

Reference computation (fp32):
    q = (x @ Wq).reshape(n, 16, 64)   # x [2048, 1024], Wq [1024, 1024]
    k = (ctx @ Wk).reshape(m, 16, 64) # ctx [2048, 768]
    v = (ctx @ Wv).reshape(m, 16, 64)
    out[h] = softmax(q[h] @ k[h].T / 8) @ v[h]
    y = out.reshape(n, 1024) @ Wo
Sharding: heads split 2-per-core (columns of Wq/Wk/Wv, rows of Wo). Each core
produces a partial y (blocked layout, bf16); the host sums the 8 partials.

v5 design (prev best 126.5us was exp/Scalar-paced at ~1067ns per mt):
  - Scores are computed transposed (scoresT [m, n]); softmax denominators come
    from a ones-column in v; exp has no max subtraction. All scores are
    pre-scaled by 128*log2e (folded into Wq host-side) so both exp paths read
    the same psum: a global 2^65 factor (B=24576 bits) cancels in softmax.
  - exp is SPLIT per-mt between two engines: Scalar runs exact ACT exp
    (scale=1/184.67, bias=65*ln2); the Vector engine runs EXP2_BITS_ANT, a
    custom one-pass DVE op: t = s'+B; f = mantissa-extract via AND/OR bit
    masks (t in [2^14,2^15) so the exponent field is constant); out_i16 =
    t + f*(C2 - f*P2), a quadratic mantissa correction (max 0.64% rel err,
    vs 0.39% for the exact bf16 path). The int16 result IS bf16 bits.
  - The two score matmuls (K=64 per head) are packed into concurrent PE
    row-tiles via tile_position (0,0)/(64,0), separate psum banks.
  - Normalization: reciprocal_approx_fast on the psum ones-row, gpsimd
    partition_broadcast, then one scalar_tensor_tensor (psum x bcast -> bf16
    oT) -- no separate evacuation copy.
  - DMA: host packs x/ctx into [piece, 128, k, 512] blobs so each dma_start
    moves 0.75-1MiB with 6-8KB contiguous per-partition runs (~300GB/s vs
    ~60GB/s for the old strided pieces). Inputs stream on the sync queue in
    need-order; outputs are staged per-block into a [128, 8, 512] tile and
    written with one gpsimd DMA per block.
  - PSUM: score ping-pong 2x2 banks + PV 2 + aux(v/proj) 1 + emit(kT/qT) 1.
"""

import os
import sys

for _p in ("/opt/trn_rl_repo", "/root/.axon_site/_ro/trn_rl_repo"):
    if os.path.isdir(_p) and _p not in sys.path:
        sys.path.insert(0, _p)

import numpy as np
import ml_dtypes

import concourse.bass as bass
import concourse.mybir as mybir
import concourse.tile as tile
from concourse import bacc
from concourse.bass_utils import run_bass_kernel_spmd

P = 128
N_TOK = 2048
M_TOK = 2048
D = 1024
C = 768
DH = 64
NB = 512
BLOCKS = [(0, 512), (512, 512), (1024, 512), (1536, 512)]
DK = D // P   # 8 x-contraction chunks
CK = C // P   # 6 ctx-contraction chunks
MT = M_TOK // P  # 16 context chunks
NPC = 4       # 512-wide DMA pieces per tensor
AT_LEAD = 2   # PV trails scores by this many mt iterations

# ---- exp scaling: score psum s' = s * 128*log2e; at = 2^65 * e^s ----
L2E = 1.4426950408889634
A_SCALE = 128.0 * L2E            # folded into Wq on host
ACT_SCALE = 1.0 / A_SCALE        # Scalar ACT: exp(s'/A + 65 ln2)
ACT_BIAS = 65.0 * float(np.log(2.0))
# EXP2_BITS_ANT constants (fit offline, see docstring)
EXP2_B = 24576.506538311176
EXP2_C2 = -5743.76031008344
EXP2_P2 = -718603.0723100811
EXP2_MASK = float(np.int32(0x0000FFFF).view(np.float32))  # mantissa-low mask

# per-16-mt engine assignment: which mts the Vector engine exps (rest Scalar)
N_VMT = int(os.environ.get("CA_NVMT", "0"))
V_SET = set(np.linspace(0, MT - 1, N_VMT, dtype=int).tolist()) if N_VMT else set()
PACK_SCORES = os.environ.get("CA_PACK", "1") == "1"
EXP_OP_MODE = os.environ.get("CA_EXPOP", "exp2")

DT = mybir.dt.bfloat16
NP_DT = ml_dtypes.bfloat16

# ---- register the custom DVE op (one pass, 8 ALU slices) ----
_EXP2_OP = None


def _register_exp2_op():
    global _EXP2_OP
    if _EXP2_OP is not None:
        return _EXP2_OP
    from concourse import dve_ops
    from concourse.dve_spec import Spec, Bin, Src0, Src1, C0, C1, C2, One, lower
    from concourse.dve_spec import AluOp
    from concourse.dve_uop import DveOpSpec

    _t = Src0 + C0
    _a = Bin(AluOp.BITWISE_AND, _t, C1)
    _b = Bin(AluOp.BITWISE_OR, _a, One)
    _w = _b - One
    _e = _w * Src1
    _h = C2 - _e
    _body = _t + (_w * _h)

    def _ref(in0, in1, s0, s1, imm2):
        t = (in0.astype(np.float32) + np.float32(s0)).astype(np.float32)
        mask = np.float32(s1).view(np.int32)
        a = (t.view(np.int32) & mask).view(np.float32)
        b = (a.view(np.int32) | np.float32(1.0).view(np.int32)).view(np.float32)
        w = (b - np.float32(1.0)).astype(np.float32)
        e = (w * in1.astype(np.float32)).astype(np.float32)
        h = (np.float32(imm2) - e).astype(np.float32)
        return t + w * h

    name = "EXP2_BITS_ANT"
    spec = Spec(body=_body, reference=_ref)
    if name not in dve_ops._SUB_OPCODE_FOR_NAME:
        row = max(dve_ops._SUB_OPCODE_FOR_NAME.values()) + 1
        assert row < 0x20
        dve_ops._SUB_OPCODE_FOR_NAME[name] = row
    shas = {}
    for ver in ("v3", "v4"):
        uops = lower(spec, ver=ver)
        shas[ver] = DveOpSpec(
            name=name,
            opcode=dve_ops._SUB_OPCODE_FOR_NAME[name],
            uops=uops,
            rd1_en=True,
        ).sha(ver)
    op = dve_ops.DveOp(name, spec, subdim=False, uops_sha=shas)
    if all(o.name != name for o in dve_ops.OPS):
        dve_ops.OPS.append(op)
    dve_ops.CUSTOM_DVE_SPECS[name] = spec
    _EXP2_OP = op
    return op


def build_core_program():
    f32 = mybir.dt.float32
    exp2_op = _register_exp2_op()

    nc = bacc.Bacc("TRN2", target_bir_lowering=False, debug=False)

    # host-packed blobs: [piece, 128, contraction-chunk, 512]
    xb = nc.declare_dram_parameter("xb", [NPC, P, DK, NB], DT, isOutput=False)
    cb = nc.declare_dram_parameter("cb", [NPC, P, CK, NB], DT, isOutput=False)
    wq = nc.declare_dram_parameter("wq", [P, DK, P], DT, isOutput=False)
    wk = nc.declare_dram_parameter("wk", [P, CK, P], DT, isOutput=False)
    wv = nc.declare_dram_parameter("wv", [P, CK, P], DT, isOutput=False)
    wo = nc.declare_dram_parameter("wo", [P, D], DT, isOutput=False)
    # output blob: [block, 128, slab, 512]; y[n0+c, s*128+p] = yO[b, p, s, c]
    yO = nc.declare_dram_parameter("yO", [len(BLOCKS), P, 8, NB], DT, isOutput=True)

    with tile.TileContext(nc) as tc:
        with (
            tc.tile_pool(name="wts", bufs=1) as wts,
            tc.tile_pool(name="att", bufs=4) as att,
            tc.tile_pool(name="yout", bufs=2) as yout,
            tc.tile_pool(name="small", bufs=2) as small,
            tc.tile_pool(name="ps_sc", bufs=2, space="PSUM") as ps_sc,   # 2x2
            tc.tile_pool(name="ps_pv", bufs=2, space="PSUM") as ps_pv,   # 2x1
            tc.tile_pool(name="ps_aux", bufs=1, space="PSUM") as ps_aux,  # 1
            tc.tile_pool(name="ps_emit", bufs=1, space="PSUM") as ps_emit,  # 1
        ):
            # ---- input DMA in need-order, all on the sync (HWDGE) queue ----
            wk_sb = wts.tile([P, CK, P], DT)
            nc.sync.dma_start(wk_sb[:], wk.ap())
            xT_sb = wts.tile([P, NPC, DK, NB], DT)
            ctx_sb = wts.tile([P, NPC, CK, NB], DT)
            nc.sync.dma_start(ctx_sb[:, 0], cb.ap()[0])
            wq_sb = wts.tile([P, DK, P], DT)
            nc.sync.dma_start(wq_sb[:], wq.ap())
            nc.sync.dma_start(xT_sb[:, 0], xb.ap()[0])
            wv_sb = wts.tile([P, CK, P], DT)
            nc.sync.dma_start(wv_sb[:], wv.ap())

            # ACT exp-table preload hides under the input DMA
            warm = small.tile([1, 8], f32, tag="warm", bufs=1)
            nc.vector.memset(warm[:], 0.0)
            nc.scalar.activation(warm[:], warm[:], mybir.ActivationFunctionType.Exp)

            nc.sync.dma_start(ctx_sb[:, 1], cb.ap()[1])
            nc.sync.dma_start(xT_sb[:, 1], xb.ap()[1])
            nc.sync.dma_start(ctx_sb[:, 2], cb.ap()[2])
            nc.sync.dma_start(ctx_sb[:, 3], cb.ap()[3])
            nc.sync.dma_start(xT_sb[:, 2], xb.ap()[2])
            nc.sync.dma_start(xT_sb[:, 3], xb.ap()[3])
            wo_sb = wts.tile([P, D], DT)
            nc.sync.dma_start(wo_sb[:], wo.ap())

            # ---- persistent intermediates ----
            kT_sb = wts.tile([P, M_TOK], DT)   # [dh(2 heads), m]
            qT_sb = wts.tile([P, N_TOK], DT)   # [dq(2 heads), n]
            # v layout [m, mt, head, 128]: col 0 = ones (softmax denominator
            # lands on psum partition 0), cols 64..127 = v values
            vAB = wts.tile([P, MT, 2, P], DT)
            oT_sb = wts.tile([P, N_TOK], DT)   # attn out^T, both heads
            p2c = wts.tile([P, 1], f32)        # EXP2 Src1 per-partition const
            nc.vector.memset(p2c[:], EXP2_P2)
            abias = wts.tile([P, 1], f32)      # ACT exp bias (65*ln2)
            nc.vector.memset(abias[:], ACT_BIAS)
            z1c = wts.tile([P, 1], f32)
            nc.vector.memset(z1c[:], 0.0)

            nc.vector.memset(vAB[:], 0.0)
            nc.vector.memset(vAB[:, :, :, 0:1], 1.0)

            def mm(out, lhsT, rhs, start, stop, tpos=None):
                nc.tensor.matmul(
                    out, lhsT, rhs, start=start, stop=stop, tile_position=tpos
                )

            # ---- emission helpers ----
            def emit_kT_group(g):
                # kT for one 512-wide m group: 6 accumulating N=512 matmuls
                ps = ps_emit.tile([P, NB], f32, tag="emit", name="ps_kg")
                for ck in range(CK):
                    mm(ps, wk_sb[:, ck, :], ctx_sb[:, g, ck, :],
                       start=(ck == 0), stop=(ck == CK - 1))
                nc.vector.tensor_copy(kT_sb[:, g * NB:(g + 1) * NB], ps)

            emit_ps = {}

            def qT_step(j, lo, hi):
                n0, w = BLOCKS[j]
                if lo == 0:
                    emit_ps[j] = ps_emit.tile([P, NB], f32, tag="emit",
                                              name=f"ps_q{j}")
                ps = emit_ps[j][:, :w]
                for c in range(lo, hi):
                    mm(ps, wq_sb[:, c, :], xT_sb[:, j, c, :w],
                       start=(c == 0), stop=(c == DK - 1))
                if hi == DK:
                    nc.vector.tensor_copy(qT_sb[:, n0:n0 + w], ps)

            def emit_v(mt):
                g, off = divmod(mt, 4)
                ps = ps_aux.tile([P, NB], f32, tag="aux", name="ps_v")[:, :P]
                for ck in range(CK):
                    mm(ps, ctx_sb[:, g, ck, off * P:(off + 1) * P], wv_sb[:, ck, :],
                       start=(ck == 0), stop=(ck == CK - 1))
                # one fused copy: [128, 2, 64] into both heads' value slots
                nc.vector.tensor_copy(
                    vAB[:, mt, :, DH:P],
                    ps.rearrange("p (h d) -> p h d", h=2),
                )

            def emit_proj(s, j, eng, force_split=False, alt=False):
                # one 128-row slab of y for block j into the staging tile;
                # alt=True ping-pongs between the aux and emit banks so the
                # next slab's matmul overlaps this slab's copy
                n0, w = BLOCKS[j]
                pool = ps_emit if (alt and s % 2 == 1) else ps_aux
                tag = "emit" if (alt and s % 2 == 1) else "aux"
                ps = pool.tile([P, NB], f32, tag=tag, name="ps_proj")[:, :w]
                mm(ps, wo_sb[:, s * P:(s + 1) * P], oT_sb[:, n0:n0 + w],
                   start=True, stop=True)
                if eng is nc.scalar and (N_VMT > 0 or force_split):
                    nc.scalar.copy(ystage[j][:, s, :w], ps)
                else:
                    nc.vector.tensor_copy(ystage[j][:, s, :w], ps)

            ystage = {}

            def exp_tile(mt, sc, at, w):
                if mt in V_SET:
                    # custom op writes the bf16-bits VALUE as f32 (custom-DVE
                    # int16 output hangs the engine); a native tensor_scalar
                    # does the f32 -> int16 RNE convert.
                    esc = small.tile([P, 2 * NB], f32, tag="esc", name="esc",
                                     bufs=2)
                    nc.vector._custom_dve(
                        exp2_op,
                        out=esc[:, : 2 * w],
                        in0=sc[:, :, :w].rearrange("p h n -> p (h n)"),
                        in1=p2c[:],
                        s0=EXP2_B,
                        s1=EXP2_MASK,
                        imm2=EXP2_C2,
                    )
                    nc.vector.tensor_scalar(
                        at[:, :, :w].rearrange("p h n -> p (h n)"),
                        esc[:, : 2 * w],
                        0.0, None, mybir.AluOpType.add,
                    )
                else:
                    nc.scalar.activation(
                        at[:, :, :w], sc[:, :, :w],
                        mybir.ActivationFunctionType.Exp,
                        bias=abias[:], scale=ACT_SCALE,
                    )

            def normalize_pair(pvA, pvB, nsl, w):
                # both recips first (V), then both broadcasts (G), then both
                # muls (V) -- the two chains overlap across engines
                rcs, bcss = [], []
                for pv, tag in ((pvA, "bcsA"), (pvB, "bcsB")):
                    rcf = small.tile([1, NB], f32, tag="recip", name="rcf")
                    nc.vector.reciprocal_approx_fast(rcf[:, :w], pv[0:1, :w])
                    rcs.append(rcf)
                for rcf, tag in zip(rcs, ("bcsA", "bcsB")):
                    bcs = small.tile([DH, NB], f32, tag=tag, name="bcs")
                    nc.gpsimd.partition_broadcast(bcs[:, :w], rcf[:, :w])
                    bcss.append(bcs)
                for h, (pv, bcs) in enumerate(zip((pvA, pvB), bcss)):
                    nc.vector.scalar_tensor_tensor(
                        oT_sb[h * DH:(h + 1) * DH, nsl],
                        pv[DH:P, :w], 1.0, bcs[:, :w],
                        op0=mybir.AluOpType.mult, op1=mybir.AluOpType.mult,
                    )

            # per-(nb, mt) extra PE work
            extras = {}

            def add_extra(nb, mt, fn):
                extras.setdefault((nb, mt), []).append(fn)

            # block 0: JIT kT group g at mt=4(g-1) (group 0 in prologue);
            # qT(1) spread over mts 12..15 once the emit bank is free
            for g in (1, 2, 3):
                add_extra(0, 4 * (g - 1), lambda g=g: emit_kT_group(g))
            for i, (lo, hi) in enumerate(((0, 2), (2, 4), (4, 6), (6, 8))):
                add_extra(0, 12 + i, lambda lo=lo, hi=hi: qT_step(1, lo, hi))
            # blocks 1-2 produce qT(nb+1) spread over mts 8..11
            for nbb in (1, 2):
                for i, (lo, hi) in enumerate(((0, 2), (2, 4), (4, 6), (6, 8))):
                    add_extra(nbb, 8 + i,
                              lambda j=nbb + 1, lo=lo, hi=hi: qT_step(j, lo, hi))
            # blocks 1-3 run the previous block's Wo projection at mts 6..13
            for nbb in (1, 2, 3):
                for s in range(8):
                    eng = nc.scalar if s % 2 == 0 else nc.vector
                    add_extra(nbb, 6 + s,
                              lambda s=s, j=nbb - 1, e=eng, a=(nbb == 3):
                              emit_proj(s, j, e, alt=a))

            # ---- prologue: kT group 0, then qT(0) ----
            with nc.named_scope("prologue"):
                emit_kT_group(0)
                qT_step(0, 0, DK)

            # ---- attention blocks ----
            def emit_pv(pvA, pvB, at, j, w):
                st, sp = (j == 0), (j == MT - 1)
                atA, atB = at[:, 0, :w], at[:, 1, :w]
                if at.dtype != DT:
                    atA, atB = atA.bitcast(DT), atB.bitcast(DT)
                mm(pvA[:, :w], vAB[:, j, 0, :], atA, start=st, stop=sp)
                mm(pvB[:, :w], vAB[:, j, 1, :], atB, start=st, stop=sp)

            for nb, (n0, w) in enumerate(BLOCKS):
                nsl = slice(n0, n0 + w)
                last = nb == len(BLOCKS) - 1
                ystage[nb] = yout.tile([P, 8, NB], DT, tag="yout",
                                       name=f"yst{nb}")
                with nc.named_scope(f"att{nb}"):
                    pvA = ps_pv.tile([P, NB], f32, tag="pv", name="pvA")
                    pvB = ps_pv.tile([P, NB], f32, tag="pv", name="pvB")
                    at_ring = {}
                    for mt in range(MT):
                        msl = slice(mt * P, (mt + 1) * P)
                        sc = ps_sc.tile([P, 2, NB], f32, tag="sc", name="sc")
                        tpA = (0, 0) if PACK_SCORES else None
                        tpB = (64, 0) if PACK_SCORES else None
                        mm(sc[:, 0, :w], kT_sb[0:DH, msl], qT_sb[0:DH, nsl],
                           start=True, stop=True, tpos=tpA)
                        mm(sc[:, 1, :w], kT_sb[DH:P, msl], qT_sb[DH:P, nsl],
                           start=True, stop=True, tpos=tpB)
                        at_dt = mybir.dt.int16 if mt in V_SET else DT
                        at = att.tile([P, 2, NB], at_dt, tag="at", name="at")
                        exp_tile(mt, sc, at, w)
                        at_ring[mt] = at
                        if nb == 0:
                            emit_v(mt)
                        for fn in extras.get((nb, mt), ()):
                            fn()
                        j = mt - AT_LEAD
                        if j >= 0:
                            emit_pv(pvA, pvB, at_ring.pop(j), j, w)
                    for j in range(MT - AT_LEAD, MT):
                        emit_pv(pvA, pvB, at_ring.pop(j), j, w)
                    normalize_pair(pvA, pvB, nsl, w)
                # block nb-1's projection ran inside block nb (extras);
                # issue its output DMA now
                if nb >= 1:
                    nc.sync.dma_start(yO.ap()[nb - 1], ystage[nb - 1][:])
                if last:
                    with nc.named_scope("tail"):
                        for s in range(8):
                            eng = nc.scalar if s % 2 == 0 else nc.vector
                            emit_proj(s, nb, eng, force_split=True, alt=True)
                            if s % 2 == 1:
                                nc.sync.dma_start(
                                    yO.ap()[nb, :, s - 1:s + 1],
                                    ystage[nb][:, s - 1:s + 1],
                                )

    nc.compile()
    return nc


_NC_CACHE = {}


def _get_nc():
    key = "v5"
    if key not in _NC_CACHE:
        _NC_CACHE[key] = build_core_program()
    return _NC_CACHE[key]


def _shuffle_w(w):
    # [o*P + p, e] -> [p, o, e] so each SBUF partition's rows are contiguous
    o_n = w.shape[0] // P
    return np.ascontiguousarray(
        w.reshape(o_n, P, w.shape[1]).transpose(1, 0, 2)
    )


def _prep_in_maps(x, ctx, Wq, Wk, Wv, Wo):
    # x [n, d] -> xb [piece, p, dk, 512]: xb[g, p, k, c] = x[g*512+c, k*128+p]
    xb = np.ascontiguousarray(
        x.reshape(NPC, NB, DK, P).transpose(0, 3, 2, 1)
    ).astype(NP_DT)
    cbb = np.ascontiguousarray(
        ctx.reshape(NPC, NB, CK, P).transpose(0, 3, 2, 1)
    ).astype(NP_DT)
    Wq_s = (Wq * (A_SCALE / 8.0)).astype(np.float32)
    in_maps = []
    for cc in range(8):
        csl = slice(cc * P, (cc + 1) * P)
        in_maps.append(
            {
                "xb": xb,
                "cb": cbb,
                "wq": _shuffle_w(np.ascontiguousarray(Wq_s[:, csl])).astype(NP_DT),
                "wk": _shuffle_w(np.ascontiguousarray(Wk[:, csl])).astype(NP_DT),
                "wv": _shuffle_w(np.ascontiguousarray(Wv[:, csl])).astype(NP_DT),
                "wo": np.ascontiguousarray(Wo[csl, :]).astype(NP_DT),
            }
        )
    return in_maps


def run(x, ctx, Wq, Wk, Wv, Wo, trace=False):
    nc = _get_nc()
    in_maps = _prep_in_maps(x, ctx, Wq, Wk, Wv, Wo)
    res = run_bass_kernel_spmd(nc, in_maps, core_ids=list(range(8)), trace=trace)
    acc = np.zeros((N_TOK, D), np.float32)
    for r in res.results:
        yb = np.asarray(r["yO"], dtype=np.float32)  # [4, 128, 8, 512]
        acc += yb.transpose(0, 3, 2, 1).reshape(N_TOK, D)
    return acc, res


def kernel(x, ctx, Wq, Wk, Wv, Wo):
    x = np.asarray(x, dtype=np.float32)
    ctx = np.asarray(ctx, dtype=np.float32)
    Wq = np.asarray(Wq, dtype=np.float32)
    Wk = np.asarray(Wk, dtype=np.float32)
    Wv = np.asarray(Wv, dtype=np.float32)
    Wo = np.asarray(Wo, dtype=np.float32)
    y, _ = run(x, ctx, Wq, Wk, Wv, Wo, trace=False)
    return y


# revision 23
# speedup vs baseline: 1.1680x; 1.0038x over previous
"""Trainium2 Bass kernel for 16-head cross attention, tensor-parallel over 8 cores.

Reference computation (fp32):
    q = (x @ Wq).reshape(n, 16, 64)   # x [2048, 1024], Wq [1024, 1024]
    k = (ctx @ Wk).reshape(m, 16, 64) # ctx [2048, 768]
    v = (ctx @ Wv).reshape(m, 16, 64)
    out[h] = softmax(q[h] @ k[h].T / 8) @ v[h]
    y = out.reshape(n, 1024) @ Wo
Sharding: heads split 2-per-core (columns of Wq/Wk/Wv, rows of Wo). Each core
produces a partial y (blocked layout, bf16); the host sums the 8 partials.

v5 design (prev best 126.5us was exp/Scalar-paced at ~1067ns per mt):
  - Scores are computed transposed (scoresT [m, n]); softmax denominators come
    from a ones-column in v; exp has no max subtraction. All scores are
    pre-scaled by 128*log2e (folded into Wq host-side) so both exp paths read
    the same psum: a global 2^65 factor (B=24576 bits) cancels in softmax.
  - exp is SPLIT per-mt between two engines: Scalar runs exact ACT exp
    (scale=1/184.67, bias=65*ln2); the Vector engine runs EXP2_BITS_ANT, a
    custom one-pass DVE op: t = s'+B; f = mantissa-extract via AND/OR bit
    masks (t in [2^14,2^15) so the exponent field is constant); out_i16 =
    t + f*(C2 - f*P2), a quadratic mantissa correction (max 0.64% rel err,
    vs 0.39% for the exact bf16 path). The int16 result IS bf16 bits.
  - The two score matmuls (K=64 per head) are packed into concurrent PE
    row-tiles via tile_position (0,0)/(64,0), separate psum banks.
  - Normalization: reciprocal_approx_fast on the psum ones-row, gpsimd
    partition_broadcast, then one scalar_tensor_tensor (psum x bcast -> bf16
    oT) -- no separate evacuation copy.
  - DMA: host packs x/ctx into [piece, 128, k, 512] blobs so each dma_start
    moves 0.75-1MiB with 6-8KB contiguous per-partition runs (~300GB/s vs
    ~60GB/s for the old strided pieces). Inputs stream on the sync queue in
    need-order; outputs are staged per-block into a [128, 8, 512] tile and
    written with one gpsimd DMA per block.
  - PSUM: score ping-pong 2x2 banks + PV 2 + aux(v/proj) 1 + emit(kT/qT) 1.
"""

import os
import sys

for _p in ("/opt/trn_rl_repo", "/root/.axon_site/_ro/trn_rl_repo"):
    if os.path.isdir(_p) and _p not in sys.path:
        sys.path.insert(0, _p)

import numpy as np
import ml_dtypes

import concourse.bass as bass
import concourse.mybir as mybir
import concourse.tile as tile
from concourse import bacc
from concourse.bass_utils import run_bass_kernel_spmd

P = 128
N_TOK = 2048
M_TOK = 2048
D = 1024
C = 768
DH = 64
NB = 512
BLOCKS = [(0, 512), (512, 512), (1024, 512), (1536, 512)]
DK = D // P   # 8 x-contraction chunks
CK = C // P   # 6 ctx-contraction chunks
MT = M_TOK // P  # 16 context chunks
NPC = 4       # 512-wide DMA pieces per tensor
AT_LEAD = 2   # PV trails scores by this many mt iterations

# ---- exp scaling: score psum s' = s * 128*log2e; at = 2^65 * e^s ----
L2E = 1.4426950408889634
A_SCALE = 128.0 * L2E            # folded into Wq on host
ACT_SCALE = 1.0 / A_SCALE        # Scalar ACT: exp(s'/A + 65 ln2)
ACT_BIAS = 65.0 * float(np.log(2.0))
# EXP2_BITS_ANT constants (fit offline, see docstring)
EXP2_B = 24576.506538311176
EXP2_C2 = -5743.76031008344
EXP2_P2 = -718603.0723100811
EXP2_MASK = float(np.int32(0x0000FFFF).view(np.float32))  # mantissa-low mask

# per-16-mt engine assignment: which mts the Vector engine exps (rest Scalar)
N_VMT = int(os.environ.get("CA_NVMT", "0"))
V_SET = set(np.linspace(0, MT - 1, N_VMT, dtype=int).tolist()) if N_VMT else set()
PACK_SCORES = os.environ.get("CA_PACK", "1") == "1"
EXP_OP_MODE = os.environ.get("CA_EXPOP", "exp2")

DT = mybir.dt.bfloat16
NP_DT = ml_dtypes.bfloat16

# ---- register the custom DVE op (one pass, 8 ALU slices) ----
_EXP2_OP = None


def _register_exp2_op():
    global _EXP2_OP
    if _EXP2_OP is not None:
        return _EXP2_OP
    from concourse import dve_ops
    from concourse.dve_spec import Spec, Bin, Src0, Src1, C0, C1, C2, One, lower
    from concourse.dve_spec import AluOp
    from concourse.dve_uop import DveOpSpec

    _t = Src0 + C0
    _a = Bin(AluOp.BITWISE_AND, _t, C1)
    _b = Bin(AluOp.BITWISE_OR, _a, One)
    _w = _b - One
    _e = _w * Src1
    _h = C2 - _e
    _body = _t + (_w * _h)

    def _ref(in0, in1, s0, s1, imm2):
        t = (in0.astype(np.float32) + np.float32(s0)).astype(np.float32)
        mask = np.float32(s1).view(np.int32)
        a = (t.view(np.int32) & mask).view(np.float32)
        b = (a.view(np.int32) | np.float32(1.0).view(np.int32)).view(np.float32)
        w = (b - np.float32(1.0)).astype(np.float32)
        e = (w * in1.astype(np.float32)).astype(np.float32)
        h = (np.float32(imm2) - e).astype(np.float32)
        return t + w * h

    name = "EXP2_BITS_ANT"
    spec = Spec(body=_body, reference=_ref)
    if name not in dve_ops._SUB_OPCODE_FOR_NAME:
        row = max(dve_ops._SUB_OPCODE_FOR_NAME.values()) + 1
        assert row < 0x20
        dve_ops._SUB_OPCODE_FOR_NAME[name] = row
    shas = {}
    for ver in ("v3", "v4"):
        uops = lower(spec, ver=ver)
        shas[ver] = DveOpSpec(
            name=name,
            opcode=dve_ops._SUB_OPCODE_FOR_NAME[name],
            uops=uops,
            rd1_en=True,
        ).sha(ver)
    op = dve_ops.DveOp(name, spec, subdim=False, uops_sha=shas)
    if all(o.name != name for o in dve_ops.OPS):
        dve_ops.OPS.append(op)
    dve_ops.CUSTOM_DVE_SPECS[name] = spec
    _EXP2_OP = op
    return op


def build_core_program():
    f32 = mybir.dt.float32
    exp2_op = _register_exp2_op()

    nc = bacc.Bacc("TRN2", target_bir_lowering=False, debug=False)

    # host-packed blobs: [piece, 128, contraction-chunk, 512]
    xb = nc.declare_dram_parameter("xb", [NPC, P, DK, NB], DT, isOutput=False)
    cb = nc.declare_dram_parameter("cb", [NPC, P, CK, NB], DT, isOutput=False)
    wq = nc.declare_dram_parameter("wq", [P, DK, P], DT, isOutput=False)
    wk = nc.declare_dram_parameter("wk", [P, CK, P], DT, isOutput=False)
    wv = nc.declare_dram_parameter("wv", [P, CK, P], DT, isOutput=False)
    wo = nc.declare_dram_parameter("wo", [P, D], DT, isOutput=False)
    # output blob: [block, 128, slab, 512]; y[n0+c, s*128+p] = yO[b, p, s, c]
    yO = nc.declare_dram_parameter("yO", [len(BLOCKS), P, 8, NB], DT, isOutput=True)

    with tile.TileContext(nc) as tc:
        with (
            tc.tile_pool(name="wts", bufs=1) as wts,
            tc.tile_pool(name="att", bufs=4) as att,
            tc.tile_pool(name="yout", bufs=2) as yout,
            tc.tile_pool(name="small", bufs=2) as small,
            tc.tile_pool(name="ps_sc", bufs=2, space="PSUM") as ps_sc,   # 2x2
            tc.tile_pool(name="ps_pv", bufs=2, space="PSUM") as ps_pv,   # 2x1
            tc.tile_pool(name="ps_aux", bufs=1, space="PSUM") as ps_aux,  # 1
            tc.tile_pool(name="ps_emit", bufs=1, space="PSUM") as ps_emit,  # 1
        ):
            # ---- input DMA in need-order, all on the sync (HWDGE) queue ----
            wk_sb = wts.tile([P, CK, P], DT)
            nc.sync.dma_start(wk_sb[:], wk.ap())
            xT_sb = wts.tile([P, NPC, DK, NB], DT)
            ctx_sb = wts.tile([P, NPC, CK, NB], DT)
            nc.sync.dma_start(ctx_sb[:, 0], cb.ap()[0])
            wq_sb = wts.tile([P, DK, P], DT)
            nc.sync.dma_start(wq_sb[:], wq.ap())
            nc.sync.dma_start(xT_sb[:, 0], xb.ap()[0])
            wv_sb = wts.tile([P, CK, P], DT)
            nc.sync.dma_start(wv_sb[:], wv.ap())

            # ACT exp-table preload hides under the input DMA
            warm = small.tile([1, 8], f32, tag="warm", bufs=1)
            nc.vector.memset(warm[:], 0.0)
            nc.scalar.activation(warm[:], warm[:], mybir.ActivationFunctionType.Exp)

            nc.sync.dma_start(ctx_sb[:, 1], cb.ap()[1])
            nc.sync.dma_start(xT_sb[:, 1], xb.ap()[1])
            nc.sync.dma_start(ctx_sb[:, 2], cb.ap()[2])
            nc.sync.dma_start(ctx_sb[:, 3], cb.ap()[3])
            nc.sync.dma_start(xT_sb[:, 2], xb.ap()[2])
            nc.sync.dma_start(xT_sb[:, 3], xb.ap()[3])
            wo_sb = wts.tile([P, D], DT)
            nc.sync.dma_start(wo_sb[:], wo.ap())

            # ---- persistent intermediates ----
            kT_sb = wts.tile([P, M_TOK], DT)   # [dh(2 heads), m]
            qT_sb = wts.tile([P, N_TOK], DT)   # [dq(2 heads), n]
            # v layout [m, mt, head, 128]: col 0 = ones (softmax denominator
            # lands on psum partition 0), cols 64..127 = v values
            vAB = wts.tile([P, MT, 2, P], DT)
            oT_sb = wts.tile([P, N_TOK], DT)   # attn out^T, both heads
            p2c = wts.tile([P, 1], f32)        # EXP2 Src1 per-partition const
            nc.vector.memset(p2c[:], EXP2_P2)
            abias = wts.tile([P, 1], f32)      # ACT exp bias (65*ln2)
            nc.vector.memset(abias[:], ACT_BIAS)
            z1c = wts.tile([P, 1], f32)
            nc.vector.memset(z1c[:], 0.0)

            nc.vector.memset(vAB[:, :, :, 0:2], 0.0)
            nc.vector.memset(vAB[:, :, :, 0:1], 1.0)

            def mm(out, lhsT, rhs, start, stop, tpos=None):
                nc.tensor.matmul(
                    out, lhsT, rhs, start=start, stop=stop, tile_position=tpos
                )

            # ---- emission helpers ----
            def emit_kT_group(g):
                # kT for one 512-wide m group: 6 accumulating N=512 matmuls
                ps = ps_emit.tile([P, NB], f32, tag="emit", name="ps_kg")
                for ck in range(CK):
                    mm(ps, wk_sb[:, ck, :], ctx_sb[:, g, ck, :],
                       start=(ck == 0), stop=(ck == CK - 1))
                nc.vector.tensor_copy(kT_sb[:, g * NB:(g + 1) * NB], ps)

            emit_ps = {}
            kt_ps = {}

            def kT_step(g, lo, hi):
                if lo == 0:
                    kt_ps[g] = ps_emit.tile([P, NB], f32, tag="emit",
                                            name=f"ps_k{g}")
                ps = kt_ps[g]
                for ck in range(lo, hi):
                    mm(ps, wk_sb[:, ck, :], ctx_sb[:, g, ck, :],
                       start=(ck == 0), stop=(ck == CK - 1))
                if hi == CK:
                    nc.vector.tensor_copy(kT_sb[:, g * NB:(g + 1) * NB], ps)

            def qT_step(j, lo, hi):
                n0, w = BLOCKS[j]
                if lo == 0:
                    emit_ps[j] = ps_emit.tile([P, NB], f32, tag="emit",
                                              name=f"ps_q{j}")
                ps = emit_ps[j][:, :w]
                for c in range(lo, hi):
                    mm(ps, wq_sb[:, c, :], xT_sb[:, j, c, :w],
                       start=(c == 0), stop=(c == DK - 1))
                if hi == DK:
                    nc.vector.tensor_copy(qT_sb[:, n0:n0 + w], ps)

            def emit_v(mt):
                g, off = divmod(mt, 4)
                ps = ps_aux.tile([P, NB], f32, tag="aux", name="ps_v")[:, :P]
                for ck in range(CK):
                    mm(ps, ctx_sb[:, g, ck, off * P:(off + 1) * P], wv_sb[:, ck, :],
                       start=(ck == 0), stop=(ck == CK - 1))
                # one fused copy: [128, 2, 64] into both heads' value slots
                nc.vector.tensor_copy(
                    vAB[:, mt, :, DH:P],
                    ps.rearrange("p (h d) -> p h d", h=2),
                )

            def emit_proj(s, j, eng, force_split=False, alt=False):
                # one 128-row slab of y for block j into the staging tile;
                # alt=True ping-pongs between the aux and emit banks so the
                # next slab's matmul overlaps this slab's copy
                n0, w = BLOCKS[j]
                pool = ps_emit if (alt and s % 2 == 1) else ps_aux
                tag = "emit" if (alt and s % 2 == 1) else "aux"
                ps = pool.tile([P, NB], f32, tag=tag, name="ps_proj")[:, :w]
                mm(ps, wo_sb[:, s * P:(s + 1) * P], oT_sb[:, n0:n0 + w],
                   start=True, stop=True)
                if eng is nc.scalar and (N_VMT > 0 or force_split):
                    nc.scalar.copy(ystage[j][:, s, :w], ps)
                else:
                    nc.vector.tensor_copy(ystage[j][:, s, :w], ps)

            ystage = {}

            def exp_tile(mt, sc, at, w):
                if mt in V_SET:
                    # custom op writes the bf16-bits VALUE as f32 (custom-DVE
                    # int16 output hangs the engine); a native tensor_scalar
                    # does the f32 -> int16 RNE convert.
                    esc = small.tile([P, 2 * NB], f32, tag="esc", name="esc",
                                     bufs=2)
                    nc.vector._custom_dve(
                        exp2_op,
                        out=esc[:, : 2 * w],
                        in0=sc[:, :, :w].rearrange("p h n -> p (h n)"),
                        in1=p2c[:],
                        s0=EXP2_B,
                        s1=EXP2_MASK,
                        imm2=EXP2_C2,
                    )
                    nc.vector.tensor_scalar(
                        at[:, :, :w].rearrange("p h n -> p (h n)"),
                        esc[:, : 2 * w],
                        0.0, None, mybir.AluOpType.add,
                    )
                else:
                    nc.scalar.activation(
                        at[:, :, :w], sc[:, :, :w],
                        mybir.ActivationFunctionType.Exp,
                        bias=abias[:], scale=ACT_SCALE,
                    )

            def normalize_pair(pvA, pvB, nsl, w):
                # both recips first (V), then both broadcasts (G), then both
                # muls (V) -- the two chains overlap across engines
                rcs, bcss = [], []
                for pv, tag in ((pvA, "bcsA"), (pvB, "bcsB")):
                    rcf = small.tile([1, NB], f32, tag="recip", name="rcf")
                    nc.vector.reciprocal_approx_fast(rcf[:, :w], pv[0:1, :w])
                    rcs.append(rcf)
                for rcf, tag in zip(rcs, ("bcsA", "bcsB")):
                    bcs = small.tile([DH, NB], f32, tag=tag, name="bcs")
                    nc.gpsimd.partition_broadcast(bcs[:, :w], rcf[:, :w])
                    bcss.append(bcs)
                for h, (pv, bcs) in enumerate(zip((pvA, pvB), bcss)):
                    nc.vector.scalar_tensor_tensor(
                        oT_sb[h * DH:(h + 1) * DH, nsl],
                        pv[DH:P, :w], 1.0, bcs[:, :w],
                        op0=mybir.AluOpType.mult, op1=mybir.AluOpType.mult,
                    )

            # per-(nb, mt) extra PE work
            extras = {}

            def add_extra(nb, mt, fn):
                extras.setdefault((nb, mt), []).append(fn)

            # block 0: JIT kT group g at mt=4(g-1) (group 0 in prologue);
            # qT(1) spread over mts 12..15 once the emit bank is free
            for g in (1, 2, 3):
                for i, (lo, hi) in enumerate(((0, 2), (2, 4), (4, 6))):
                    add_extra(0, 4 * (g - 1) + i,
                              lambda g=g, lo=lo, hi=hi: kT_step(g, lo, hi))
            for i, (lo, hi) in enumerate(((0, 2), (2, 4), (4, 6), (6, 8))):
                add_extra(0, 12 + i, lambda lo=lo, hi=hi: qT_step(1, lo, hi))
            # blocks 1-2 produce qT(nb+1) spread over mts 8..11
            for nbb in (1, 2):
                for i, (lo, hi) in enumerate(((0, 2), (2, 4), (4, 6), (6, 8))):
                    add_extra(nbb, 8 + i,
                              lambda j=nbb + 1, lo=lo, hi=hi: qT_step(j, lo, hi))
            # blocks 1-3 run the previous block's Wo projection at mts 6..13
            for nbb in (1, 2, 3):
                for s in range(8):
                    eng = nc.scalar if s % 2 == 0 else nc.vector
                    add_extra(nbb, 6 + s,
                              lambda s=s, j=nbb - 1, e=eng, a=(nbb == 3):
                              emit_proj(s, j, e, alt=a))

            # ---- prologue: kT group 0, then qT(0) ----
            with nc.named_scope("prologue"):
                emit_kT_group(0)
                qT_step(0, 0, DK)

            # ---- attention blocks ----
            def emit_pv(pvA, pvB, at, j, w):
                st, sp = (j == 0), (j == MT - 1)
                atA, atB = at[:, 0, :w], at[:, 1, :w]
                if at.dtype != DT:
                    atA, atB = atA.bitcast(DT), atB.bitcast(DT)
                mm(pvA[:, :w], vAB[:, j, 0, :], atA, start=st, stop=sp)
                mm(pvB[:, :w], vAB[:, j, 1, :], atB, start=st, stop=sp)

            for nb, (n0, w) in enumerate(BLOCKS):
                nsl = slice(n0, n0 + w)
                last = nb == len(BLOCKS) - 1
                ystage[nb] = yout.tile([P, 8, NB], DT, tag="yout",
                                       name=f"yst{nb}")
                with nc.named_scope(f"att{nb}"):
                    pvA = ps_pv.tile([P, NB], f32, tag="pv", name="pvA")
                    pvB = ps_pv.tile([P, NB], f32, tag="pv", name="pvB")
                    at_ring = {}
                    for mt in range(MT):
                        msl = slice(mt * P, (mt + 1) * P)
                        sc = ps_sc.tile([P, 2, NB], f32, tag="sc", name="sc")
                        tpA = (0, 0) if PACK_SCORES else None
                        tpB = (64, 0) if PACK_SCORES else None
                        mm(sc[:, 0, :w], kT_sb[0:DH, msl], qT_sb[0:DH, nsl],
                           start=True, stop=True, tpos=tpA)
                        mm(sc[:, 1, :w], kT_sb[DH:P, msl], qT_sb[DH:P, nsl],
                           start=True, stop=True, tpos=tpB)
                        at_dt = mybir.dt.int16 if mt in V_SET else DT
                        at = att.tile([P, 2, NB], at_dt, tag="at", name="at")
                        exp_tile(mt, sc, at, w)
                        at_ring[mt] = at
                        if nb == 0:
                            emit_v(mt)
                        for fn in extras.get((nb, mt), ()):
                            fn()
                        j = mt - AT_LEAD
                        if j >= 0:
                            emit_pv(pvA, pvB, at_ring.pop(j), j, w)
                    for j in range(MT - AT_LEAD, MT):
                        emit_pv(pvA, pvB, at_ring.pop(j), j, w)
                    normalize_pair(pvA, pvB, nsl, w)
                # block nb-1's projection ran inside block nb (extras);
                # issue its output DMA now
                if nb >= 1:
                    nc.sync.dma_start(yO.ap()[nb - 1], ystage[nb - 1][:])
                if last:
                    with nc.named_scope("tail"):
                        for s in range(8):
                            eng = nc.scalar if s % 2 == 0 else nc.vector
                            emit_proj(s, nb, eng, force_split=True, alt=True)
                            if s % 2 == 1:
                                nc.sync.dma_start(
                                    yO.ap()[nb, :, s - 1:s + 1],
                                    ystage[nb][:, s - 1:s + 1],
                                )

    nc.compile()
    return nc


_NC_CACHE = {}


def _get_nc():
    key = "v5"
    if key not in _NC_CACHE:
        _NC_CACHE[key] = build_core_program()
    return _NC_CACHE[key]


def _shuffle_w(w):
    # [o*P + p, e] -> [p, o, e] so each SBUF partition's rows are contiguous
    o_n = w.shape[0] // P
    return np.ascontiguousarray(
        w.reshape(o_n, P, w.shape[1]).transpose(1, 0, 2)
    )


def _prep_in_maps(x, ctx, Wq, Wk, Wv, Wo):
    # x [n, d] -> xb [piece, p, dk, 512]: xb[g, p, k, c] = x[g*512+c, k*128+p]
    xb = np.ascontiguousarray(
        x.reshape(NPC, NB, DK, P).transpose(0, 3, 2, 1)
    ).astype(NP_DT)
    cbb = np.ascontiguousarray(
        ctx.reshape(NPC, NB, CK, P).transpose(0, 3, 2, 1)
    ).astype(NP_DT)
    Wq_s = (Wq * (A_SCALE / 8.0)).astype(np.float32)
    in_maps = []
    for cc in range(8):
        csl = slice(cc * P, (cc + 1) * P)
        in_maps.append(
            {
                "xb": xb,
                "cb": cbb,
                "wq": _shuffle_w(np.ascontiguousarray(Wq_s[:, csl])).astype(NP_DT),
                "wk": _shuffle_w(np.ascontiguousarray(Wk[:, csl])).astype(NP_DT),
                "wv": _shuffle_w(np.ascontiguousarray(Wv[:, csl])).astype(NP_DT),
                "wo": np.ascontiguousarray(Wo[csl, :]).astype(NP_DT),
            }
        )
    return in_maps


def run(x, ctx, Wq, Wk, Wv, Wo, trace=False):
    nc = _get_nc()
    in_maps = _prep_in_maps(x, ctx, Wq, Wk, Wv, Wo)
    res = run_bass_kernel_spmd(nc, in_maps, core_ids=list(range(8)), trace=trace)
    acc = np.zeros((N_TOK, D), np.float32)
    for r in res.results:
        yb = np.asarray(r["yO"], dtype=np.float32)  # [4, 128, 8, 512]
        acc += yb.transpose(0, 3, 2, 1).reshape(N_TOK, D)
    return acc, res


def kernel(x, ctx, Wq, Wk, Wv, Wo):
    x = np.asarray(x, dtype=np.float32)
    ctx = np.asarray(ctx, dtype=np.float32)
    Wq = np.asarray(Wq, dtype=np.float32)
    Wk = np.asarray(Wk, dtype=np.float32)
    Wv = np.asarray(Wv, dtype=np.float32)
    Wo = np.asarray(Wo, dtype=np.float32)
    y, _ = run(x, ctx, Wq, Wk, Wv, Wo, trace=False)
    return y
